# revision 20
# baseline (speedup 1.0000x reference)
"""2-layer GCN on 8 Trainium2 NeuronCores.

Strategy (edge-cut node sharding):
- 8 cores, core c owns dst nodes [c*12500, (c+1)*12500).
- Per layer, each core builds its shard of the message table tab = (x @ W) * dinv
  (node-major, 256B f32 rows), sliced into 4 node QUARTERS of [4096,4096,4096,212]
  rows. Each quarter is AllGathered separately as soon as it is staged, so the
  table build, the collectives, and the aggregation gathers all pipeline. The
  concatenated quarter-q shards of all 8 cores form gather chunk q
  (max 32768 rows = exactly the int16 index range).
- Aggregation: edges sorted by (window-group of 4x512 dsts, chunk, window, dst).
  One dma_gather per (group, chunk) [S ~= 7.6k], alternating between TWO SWDGE
  queues so Q7 descriptor generation overlaps (the bottleneck, ~5ns/row).
  Per 128-slot tile a one-hot [128,128] is built on the DVE (is_equal vs iota)
  and the PE matmul accumulates into the owning window's feature-major PSUM
  tile [64 feats, 512 dsts]. Padding slots carry dstcol=-1 -> zero one-hot.
- Self-loop terms are accumulated directly into each window's PSUM by PE
  transpose-matmuls of the staged table tiles (tab^T @ I), so
  flush = (psum) * dinv[dst] (+ bias, + ReLU for layer 1).
- Layer-2 table tiles are computed immediately after each window flush from a
  rolling hT tile, so the layer boundary costs only the tiny quarter-3
  store + AllGather.
- SPMD: one program for all 8 cores; per-(window,chunk) slot counts are the max
  over cores (padded with row-0 gathers / dstcol=-1).
Host side does only sharding/layout/integer structure (edge sort, degree counts,
index arrays); all float math runs on device.
"""
import numpy as np

N = 100000
E = 1600000
FIN = 128
HID = 64
FOUT = 64
NCORES = 8
NSH = N // NCORES           # 12500 nodes per core
WIN = 512                   # dst nodes per PSUM window
NW = (NSH + WIN - 1) // WIN  # 25 windows
WGW = 2                      # windows per gather group
NWG = (NW + WGW - 1) // WGW  # 13 groups (last has 1 window)
NTILE_NODE = (NSH + 127) // 128  # 98 node tiles per core

QB = [0, 3584, 6656, 9728]       # quarter row starts (per-core node space)
QS = [3584, 3072, 3072, NSH - 9728]    # quarter sizes (last = 2772)
NQ = 4


def _quarter_of(r):
    return np.searchsorted(np.asarray(QB + [NSH]), r, side="right") - 1


def _preprocess(edge_index):
    """Build the common SPMD schedule + per-core index/dstcol arrays.

    Slot order: (window-group wg, chunk q, window w, dst). Each (w, q) segment
    is padded to a multiple of 128 (max count over cores), so every 128-slot
    tile belongs to exactly one window.
    """
    src = np.asarray(edge_index[0], dtype=np.int64)
    dst = np.asarray(edge_index[1], dtype=np.int64)
    deg = (np.bincount(dst, minlength=N) + 1).astype(np.float32)

    qb = np.asarray(QB, np.int64)
    qs = np.asarray(QS, np.int64)

    percore = []
    for c in range(NCORES):
        lo, hi = c * NSH, (c + 1) * NSH
        sel = (dst >= lo) & (dst < hi)
        s, d = src[sel], dst[sel] - lo
        w = d // WIN
        wg = w // WGW
        sc = s // NSH                  # owner core of src
        sr = s % NSH
        b = _quarter_of(sr)            # gather chunk
        lidx = sc * qs[b] + (sr - qb[b])   # row within chunk b
        order = np.lexsort((d, w, b, wg))
        lidx, d, w, b = lidx[order], d[order], w[order], b[order]
        cnt = np.zeros((NW, NQ), np.int64)
        np.add.at(cnt, (w, b), 1)
        percore.append((lidx, d, cnt))

    cnts = np.stack([pc[2] for pc in percore])      # [8, NW, NQ]
    S_wb = ((cnts.max(axis=0) + 127) // 128) * 128  # padded per (w, b)

    # segment start offsets in (wg, b, w) order
    seg_start = np.zeros((NW, NQ), np.int64)
    pos = 0
    for wg in range(NWG):
        ws = list(range(WGW * wg, min(WGW * wg + WGW, NW)))
        for b in range(NQ):
            for w in ws:
                seg_start[w, b] = pos
                pos += S_wb[w, b]
    total_slots = int(pos)

    gidx = np.zeros((NCORES, total_slots), np.int64)
    dcol = np.full((NCORES, total_slots), -1.0, np.float32)
    rawcol = np.full((NCORES, total_slots), -1, np.int64)
    for c in range(NCORES):
        lidx, d, cnt = percore[c]
        pos_c = 0
        for wg in range(NWG):
            ws = list(range(WGW * wg, min(WGW * wg + WGW, NW)))
            for b in range(NQ):
                for w in ws:
                    n = cnt[w, b]
                    base = seg_start[w, b]
                    gidx[c, base:base + n] = lidx[pos_c:pos_c + n]
                    rawcol[c, base:base + n] = d[pos_c:pos_c + n] - w * WIN
                    pos_c += n
        assert pos_c == len(lidx)

    dcol[rawcol >= 0] = rawcol[rawcol >= 0].astype(np.float32)

    call_S = np.zeros((NWG, NQ), np.int64)
    call_start = np.zeros((NWG, NQ), np.int64)
    o_list = []   # [NWG * NQ] -> list over tiles of (wi, [(o, wd), ...])
    for wg in range(NWG):
        ws = list(range(WGW * wg, min(WGW * wg + WGW, NW)))
        for b in range(NQ):
            call_start[wg, b] = seg_start[ws[0], b]
            call_S[wg, b] = sum(int(S_wb[w, b]) for w in ws)
            tiles = []
            for wi, w in enumerate(ws):
                base = seg_start[w, b]
                nt = int(S_wb[w, b]) // 128
                for j in range(nt):
                    seg = rawcol[:, base + j * 128: base + (j + 1) * 128]
                    real = seg[seg >= 0]
                    if real.size == 0:
                        tiles.append((wi, []))
                        continue
                    lo_c, hi_c = int(real.min()), int(real.max())
                    o1 = min(lo_c, WIN - 128)
                    sub = [(o1, min(128, WIN - o1))]
                    while hi_c >= sub[-1][0] + 128:
                        o2 = sub[-1][0] + 128
                        sub.append((o2, min(128, WIN - o2)))
                    tiles.append((wi, sub))
            o_list.append(tiles)

    def wrap16(flat):
        n = flat.shape[0]
        wtile = flat.reshape(n // 16, 16).T.astype(np.int16)
        return np.tile(wtile, (8, 1))

    gidx_w = np.stack([wrap16(gidx[c]) for c in range(NCORES)])
    dcol_b = dcol.reshape(NCORES, total_slots // 128, 128).transpose(0, 2, 1).copy()

    return deg, gidx_w, dcol_b, call_S, call_start, o_list, total_slots


def _build_program(call_S, call_start, o_list, total_slots):
    from concourse import bass, bacc, mybir, tile

    f32 = mybir.dt.float32
    nc = bacc.Bacc(None, target_bir_lowering=False, num_swdge_queues=2,
                   dynamic_dma_scratch_size=24576)

    xT = nc.dram_tensor("xT", [FIN, NSH], f32, kind="ExternalInput")
    W1 = nc.dram_tensor("W1", [FIN, HID], f32, kind="ExternalInput")
    W2 = nc.dram_tensor("W2", [HID, FOUT], f32, kind="ExternalInput")
    b1 = nc.dram_tensor("b1", [HID, 1], f32, kind="ExternalInput")
    b2 = nc.dram_tensor("b2", [FOUT, 1], f32, kind="ExternalInput")
    degT = nc.dram_tensor("deg", [128, NTILE_NODE], f32, kind="ExternalInput")
    gidxT = nc.dram_tensor("gidx", [128, total_slots // 16], mybir.dt.int16,
                           kind="ExternalInput")
    dcolT = nc.dram_tensor("dcol", [128, total_slots // 128], f32,
                           kind="ExternalInput")
    outT = nc.dram_tensor("out", [FOUT, NSH], f32, kind="ExternalOutput")

    # per-layer, per-quarter staging + gathered tables
    tmy = [[nc.dram_tensor(f"tmy{l}_{q}", [QS[q], HID], f32) for q in range(NQ)]
           for l in (0, 1)]
    tfq = [[nc.dram_tensor(f"tfq{l}_{q}", [NCORES * QS[q], HID], f32,
                           addr_space="Shared") for q in range(NQ)]
           for l in (0, 1)]
    dinv_dram = nc.dram_tensor("dinv_dram", [NTILE_NODE * 128], f32)

    iota_np = np.tile(np.arange(128, dtype=np.float32), (128, 1))
    iota_dram = nc.inline_tensor(iota_np, name="iota128")
    ident_dram = nc.inline_tensor(np.eye(128, dtype=np.float32), name="ident128")

    rg = [list(range(NCORES))]

    def store_quarter(sbtab, dst_l, q):
        """DMA sbtab columns for quarter q into tmy[dst_l][q] (node-major)."""
        t0 = QB[q] // 128
        nfull = QS[q] // 128
        if nfull:
            nc.sync.dma_start(
                out=tmy[dst_l][q].ap()[:nfull * 128, :].rearrange(
                    "(t p) f -> p t f", p=128),
                in_=sbtab[:, (t0 * HID):(t0 + nfull) * HID].rearrange(
                    "p (t f) -> p t f", t=nfull))
        rem = QS[q] - nfull * 128
        if rem:
            nc.sync.dma_start(
                out=tmy[dst_l][q].ap()[nfull * 128:, :],
                in_=sbtab[:rem, (t0 + nfull) * HID:(t0 + nfull + 1) * HID])

    def allgather_quarter(dst_l, q):
        nc.gpsimd.collective_compute(
            "AllGather", mybir.AluOpType.bypass, replica_groups=rg,
            ins=[tmy[dst_l][q].ap().opt()], outs=[tfq[dst_l][q].ap().opt()])

    with tile.TileContext(nc) as tc:
        with (
            tc.tile_pool(name="const", bufs=1) as cpool,
            tc.tile_pool(name="dinvb", bufs=1) as dbpool,
            tc.tile_pool(name="stag2", bufs=1) as spool2,
            tc.tile_pool(name="psA", bufs=2, space="PSUM") as psA,
        ):
            w1t = cpool.tile([FIN, HID], f32)
            nc.sync.dma_start(out=w1t[:], in_=W1[:, :])
            w2t = cpool.tile([HID, FOUT], f32)
            nc.sync.dma_start(out=w2t[:], in_=W2[:, :])
            b1t = cpool.tile([HID, 1], f32)
            nc.sync.dma_start(out=b1t[:], in_=b1[:, :])
            b2t = cpool.tile([FOUT, 1], f32)
            nc.sync.dma_start(out=b2t[:], in_=b2[:, :])
            iot = cpool.tile([128, 128], f32)
            nc.sync.dma_start(out=iot[:], in_=iota_dram[:, :])
            idt = cpool.tile([128, 128], f32)
            nc.sync.dma_start(out=idt[:], in_=ident_dram[:, :])
            degt = cpool.tile([128, NTILE_NODE], f32)
            nc.sync.dma_start(out=degt[:], in_=degT[:, :])
            dsq = cpool.tile([128, NTILE_NODE], f32)
            nc.scalar.activation(dsq[:], degt[:],
                                 mybir.ActivationFunctionType.Sqrt)
            dinv = cpool.tile([128, NTILE_NODE], f32)
            nc.vector.reciprocal(dinv[:], dsq[:])
            nc.sync.dma_start(
                out=dinv_dram.ap().rearrange("(t p) -> p t", p=128), in_=dinv[:])
            dinvb = dbpool.tile([HID, NSH], f32)
            nc.sync.dma_start(out=dinvb[:1, :], in_=dinv_dram.ap()[None, :NSH])
            k = 1
            while k < HID:
                kk = min(k, HID - k)
                nc.sync.dma_start(out=dinvb[k:k + kk, :], in_=dinvb[:kk, :])
                k += kk

            sbTab2 = spool2.tile([128, NTILE_NODE * HID], f32)

            with tc.tile_pool(name="stag1", bufs=1) as spool1:
                sbTab1 = spool1.tile([128, NTILE_NODE * HID], f32)

                # ---- layer-1 table: tab1[n] = (x @ W1)[n] * dinv[n] ----
                XB = 8
                with tc.tile_pool(name="xT", bufs=2) as xpool:
                    for q in range(NQ):
                        tq0 = QB[q] // 128
                        tq1 = (QB[q] + QS[q] + 127) // 128
                        for t0 in range(tq0, tq1, XB):
                            t1 = min(tq1, t0 + XB)
                            n0, n1 = t0 * 128, min(NSH, t1 * 128)
                            xt = xpool.tile([FIN, XB * 128], f32)
                            nc.sync.dma_start(out=xt[:, :n1 - n0],
                                              in_=xT[:, n0:n1])
                            for t in range(t0, t1):
                                m0 = t * 128
                                nn = min(NSH, m0 + 128) - m0
                                sl = xt[:, (m0 - n0):(m0 - n0) + nn]
                                ps = psA.tile([128, HID], f32, space="PSUM")
                                nc.tensor.matmul(ps[:nn, :], lhsT=sl,
                                                 rhs=w1t[:],
                                                 start=True, stop=True)
                                nc.vector.tensor_scalar_mul(
                                    sbTab1[:nn, t * HID:(t + 1) * HID],
                                    ps[:nn, :], dinv[:nn, t:t + 1])
                        store_quarter(sbTab1, 0, q)
                        allgather_quarter(0, q)

                # ---- aggregation layers ----
                for layer in (0, 1):
                    sbtab = sbTab1 if layer == 0 else sbTab2
                    call_idx = 0
                    with (
                        tc.tile_pool(name=f"gb{layer}", bufs=4) as gpool,
                        tc.tile_pool(name=f"ix{layer}", bufs=4) as ipool,
                        tc.tile_pool(name=f"dc{layer}", bufs=4) as dpool,
                        tc.tile_pool(name=f"oh{layer}", bufs=4) as ohpool,
                        tc.tile_pool(name=f"fl{layer}", bufs=2) as flpool,
                        tc.tile_pool(name=f"ht{layer}", bufs=2) as htpool,
                        tc.tile_pool(name=f"psW{layer}", bufs=4,
                                     space="PSUM") as psW,
                    ):
                        for wg in range(NWG):
                            if layer == 0 and wg == 5:
                                allgather_quarter(1, 0)
                            if layer == 0 and wg == 8:
                                allgather_quarter(1, 1)
                            if layer == 0 and wg == 11:
                                allgather_quarter(1, 2)
                            ws = list(range(WGW * wg, min(WGW * wg + WGW, NW)))
                            psws = []
                            for w in ws:
                                p = psW.tile([HID, WIN], f32, space="PSUM")
                                nc.vector.memset(p[:], 0.0)
                                psws.append(p)
                                # self-loop term: psw[:, cols] += tab_tile^T
                                for k in range(4):
                                    t = 4 * w + k
                                    if t >= NTILE_NODE:
                                        break
                                    nn = min(NSH, t * 128 + 128) - t * 128
                                    nc.tensor.matmul(
                                        p[:, k * 128:k * 128 + nn],
                                        lhsT=sbtab[:nn,
                                                   t * HID:(t + 1) * HID],
                                        rhs=idt[:nn, :nn],
                                        start=False, stop=True)
                            for b in range(NQ):
                                gi = wg * NQ + b
                                S = int(call_S[wg, b])
                                if S == 0:
                                    continue
                                base = int(call_start[wg, b])
                                nb = S // 128
                                it = ipool.tile([128, S // 16],
                                                mybir.dt.int16)
                                nc.scalar.dma_start(
                                    out=it[:],
                                    in_=gidxT[:, base // 16:
                                              base // 16 + S // 16])
                                dt_ = dpool.tile([128, nb], f32)
                                nc.scalar.dma_start(
                                    out=dt_[:],
                                    in_=dcolT[:, base // 128:
                                              base // 128 + nb])
                                g = gpool.tile([128, nb * HID], f32)
                                nc.gpsimd.dma_gather(
                                    g[:].rearrange("p (n f) -> p n f", n=nb),
                                    tfq[layer][b][:, :],
                                    it[:], S, S, HID, single_packet=False,
                                    queue_num=call_idx % 2)
                                call_idx += 1
                                tiles = o_list[gi]
                                for j in range(nb):
                                    wi, offs = tiles[j]
                                    for (o, wd) in offs:
                                        oh = ohpool.tile([128, 128], f32)
                                        nc.vector.scalar_tensor_tensor(
                                            out=oh[:, :wd],
                                            in0=dt_[:, j:j + 1].to_broadcast(
                                                [128, wd]),
                                            scalar=float(o),
                                            in1=iot[:, :wd],
                                            op0=mybir.AluOpType.subtract,
                                            op1=mybir.AluOpType.is_equal)
                                        nc.tensor.matmul(
                                            psws[wi][:, o:o + wd],
                                            lhsT=g[:, j * HID:(j + 1) * HID],
                                            rhs=oh[:, :wd], start=False,
                                            stop=True)
                            # flush the group's windows
                            for wi, w in enumerate(ws):
                                c0 = w * WIN
                                c1 = min(NSH, c0 + WIN)
                                ncol = c1 - c0
                                if layer == 0:
                                    htr = htpool.tile([HID, WIN], f32)
                                    nc.vector.tensor_mul(htr[:, :ncol],
                                                         psws[wi][:, :ncol],
                                                         dinvb[:, c0:c1])
                                    nc.scalar.activation(
                                        htr[:, :ncol], htr[:, :ncol],
                                        mybir.ActivationFunctionType.Relu,
                                        bias=b1t[:])
                                    # layer-2 table tiles for this window
                                    for k in range(4):
                                        t = 4 * w + k
                                        if t >= NTILE_NODE:
                                            break
                                        nn = min(NSH, t * 128 + 128) - t * 128
                                        ps = psA.tile([128, FOUT], f32,
                                                      space="PSUM")
                                        nc.tensor.matmul(
                                            ps[:nn, :],
                                            lhsT=htr[:, k * 128:k * 128 + nn],
                                            rhs=w2t[:],
                                            start=True, stop=True)
                                        nc.vector.tensor_scalar_mul(
                                            sbTab2[:nn,
                                                   t * HID:(t + 1) * HID],
                                            ps[:nn, :], dinv[:nn, t:t + 1])
                                else:
                                    fl = flpool.tile([HID, WIN], f32)
                                    nc.vector.tensor_mul(fl[:, :ncol],
                                                         psws[wi][:, :ncol],
                                                         dinvb[:, c0:c1])
                                    nc.vector.tensor_scalar_add(
                                        fl[:, :ncol], fl[:, :ncol], b2t[:])
                                    nc.sync.dma_start(out=outT[:, c0:c1],
                                                      in_=fl[:, :ncol])
                            if layer == 0 and wg in (3, 6, 9, 12):
                                # quarter fully flushed -> stage layer-2 rows
                                store_quarter(sbTab2, 1, {3: 0, 6: 1,
                                                          9: 2, 12: 3}[wg])
                        if layer == 0:
                            allgather_quarter(1, 3)
    nc.compile()
    return nc


TRACE = False        # set True (e.g. from test.py) to capture HW exec time
_LAST_TIMING = None


def kernel(x, edge_index, W1, b1, W2, b2):
    from concourse.bass_utils import run_bass_kernel_spmd

    x = np.asarray(x, np.float32)
    W1 = np.asarray(W1, np.float32)
    W2 = np.asarray(W2, np.float32)
    b1 = np.asarray(b1, np.float32)
    b2 = np.asarray(b2, np.float32)

    deg, gidx_w, dcol_b, call_S, call_start, o_list, total_slots = \
        _preprocess(edge_index)

    nc = _build_program(call_S, call_start, o_list, total_slots)

    in_maps = []
    for c in range(NCORES):
        lo, hi = c * NSH, (c + 1) * NSH
        degc = deg[lo:hi]
        degp = np.ones(NTILE_NODE * 128, np.float32)
        degp[:NSH] = degc
        in_maps.append({
            "xT": np.ascontiguousarray(x[lo:hi].T),
            "W1": W1, "W2": W2,
            "b1": b1.reshape(HID, 1), "b2": b2.reshape(FOUT, 1),
            "deg": np.ascontiguousarray(degp.reshape(NTILE_NODE, 128).T),
            "gidx": gidx_w[c],
            "dcol": dcol_b[c],
        })

    kwargs = {"trace": True} if TRACE else {}
    res = run_bass_kernel_spmd(nc, in_maps, core_ids=list(range(NCORES)),
                               **kwargs)
    globals()["_LAST_TIMING"] = getattr(res, "exec_time_ns", None)

    z = np.empty((N, FOUT), np.float32)
    for c in range(NCORES):
        lo, hi = c * NSH, (c + 1) * NSH
        z[lo:hi] = np.asarray(res.results[c]["out"]).reshape(FOUT, NSH).T
    return z


# revision 22
# speedup vs baseline: 1.1020x; 1.1020x over previous
"""2-layer GCN on 8 Trainium2 NeuronCores.

Strategy (edge-cut node sharding):
- 8 cores, core c owns dst nodes [c*12500, (c+1)*12500).
- Per layer, each core builds its shard of the message table tab = (x @ W) * dinv
  (node-major, 256B f32 rows), sliced into 4 node QUARTERS of [4096,4096,4096,212]
  rows. Each quarter is AllGathered separately as soon as it is staged, so the
  table build, the collectives, and the aggregation gathers all pipeline. The
  concatenated quarter-q shards of all 8 cores form gather chunk q
  (max 32768 rows = exactly the int16 index range).
- Aggregation: edges sorted by (window-group of 4x512 dsts, chunk, window, dst).
  One dma_gather per (group, chunk) [S ~= 7.6k], alternating between TWO SWDGE
  queues so Q7 descriptor generation overlaps (the bottleneck, ~5ns/row).
  Per 128-slot tile a one-hot [128,128] is built on the DVE (is_equal vs iota)
  and the PE matmul accumulates into the owning window's feature-major PSUM
  tile [64 feats, 512 dsts]. Padding slots carry dstcol=-1 -> zero one-hot.
- Self-loop terms are accumulated directly into each window's PSUM by PE
  transpose-matmuls of the staged table tiles (tab^T @ I), so
  flush = (psum) * dinv[dst] (+ bias, + ReLU for layer 1).
- Layer-2 table tiles are computed immediately after each window flush from a
  rolling hT tile, so the layer boundary costs only the tiny quarter-3
  store + AllGather.
- SPMD: one program for all 8 cores; per-(window,chunk) slot counts are the max
  over cores (padded with row-0 gathers / dstcol=-1).
Host side does only sharding/layout/integer structure (edge sort, degree counts,
index arrays); all float math runs on device.
"""
import numpy as np

N = 100000
E = 1600000
FIN = 128
HID = 64
FOUT = 64
NCORES = 8
NSH = N // NCORES           # 12500 nodes per core
WIN = 512                   # dst nodes per PSUM window
NW = (NSH + WIN - 1) // WIN  # 25 windows
WGW = 2                      # windows per gather group
NWG = (NW + WGW - 1) // WGW  # 13 groups (last has 1 window)
NTILE_NODE = (NSH + 127) // 128  # 98 node tiles per core

QB = [0, 3584, 6656, 9728]       # quarter row starts (per-core node space)
QS = [3584, 3072, 3072, NSH - 9728]    # quarter sizes (last = 2772)
NQ = 4


def _quarter_of(r):
    return np.searchsorted(np.asarray(QB + [NSH]), r, side="right") - 1


def _preprocess(edge_index):
    """Build the common SPMD schedule + per-core index/dstcol arrays.

    Slot order: (window-group wg, chunk q, window w, dst). Each (w, q) segment
    is padded to a multiple of 128 (max count over cores), so every 128-slot
    tile belongs to exactly one window.
    """
    src = np.asarray(edge_index[0], dtype=np.int64)
    dst = np.asarray(edge_index[1], dtype=np.int64)
    deg = (np.bincount(dst, minlength=N) + 1).astype(np.float32)

    qb = np.asarray(QB, np.int64)
    qs = np.asarray(QS, np.int64)

    percore = []
    for c in range(NCORES):
        lo, hi = c * NSH, (c + 1) * NSH
        sel = (dst >= lo) & (dst < hi)
        s, d = src[sel], dst[sel] - lo
        w = d // WIN
        wg = w // WGW
        sc = s // NSH                  # owner core of src
        sr = s % NSH
        b = _quarter_of(sr)            # gather chunk
        lidx = sc * qs[b] + (sr - qb[b])   # row within chunk b
        order = np.lexsort((d, w, b, wg))
        lidx, d, w, b = lidx[order], d[order], w[order], b[order]
        cnt = np.zeros((NW, NQ), np.int64)
        np.add.at(cnt, (w, b), 1)
        percore.append((lidx, d, cnt))

    cnts = np.stack([pc[2] for pc in percore])      # [8, NW, NQ]
    S_wb = ((cnts.max(axis=0) + 127) // 128) * 128  # padded per (w, b)

    # segment start offsets in (wg, b, w) order
    seg_start = np.zeros((NW, NQ), np.int64)
    pos = 0
    for wg in range(NWG):
        ws = list(range(WGW * wg, min(WGW * wg + WGW, NW)))
        for b in range(NQ):
            for w in ws:
                seg_start[w, b] = pos
                pos += S_wb[w, b]
    total_slots = int(pos)

    gidx = np.zeros((NCORES, total_slots), np.int64)
    dcol = np.full((NCORES, total_slots), -1.0, np.float32)
    rawcol = np.full((NCORES, total_slots), -1, np.int64)
    for c in range(NCORES):
        lidx, d, cnt = percore[c]
        pos_c = 0
        for wg in range(NWG):
            ws = list(range(WGW * wg, min(WGW * wg + WGW, NW)))
            for b in range(NQ):
                for w in ws:
                    n = cnt[w, b]
                    base = seg_start[w, b]
                    gidx[c, base:base + n] = lidx[pos_c:pos_c + n]
                    rawcol[c, base:base + n] = d[pos_c:pos_c + n] - w * WIN
                    pos_c += n
        assert pos_c == len(lidx)

    dcol[rawcol >= 0] = rawcol[rawcol >= 0].astype(np.float32)

    call_S = np.zeros((NWG, NQ), np.int64)
    call_start = np.zeros((NWG, NQ), np.int64)
    o_list = []   # [NWG * NQ] -> list over tiles of (wi, [(o, wd), ...])
    for wg in range(NWG):
        ws = list(range(WGW * wg, min(WGW * wg + WGW, NW)))
        for b in range(NQ):
            call_start[wg, b] = seg_start[ws[0], b]
            call_S[wg, b] = sum(int(S_wb[w, b]) for w in ws)
            tiles = []
            for wi, w in enumerate(ws):
                base = seg_start[w, b]
                nt = int(S_wb[w, b]) // 128
                for j in range(nt):
                    seg = rawcol[:, base + j * 128: base + (j + 1) * 128]
                    real = seg[seg >= 0]
                    if real.size == 0:
                        tiles.append((wi, []))
                        continue
                    lo_c, hi_c = int(real.min()), int(real.max())
                    o1 = min(lo_c, WIN - 128)
                    sub = [(o1, min(128, WIN - o1))]
                    while hi_c >= sub[-1][0] + 128:
                        o2 = sub[-1][0] + 128
                        sub.append((o2, min(128, WIN - o2)))
                    tiles.append((wi, sub))
            o_list.append(tiles)

    def wrap16(flat):
        n = flat.shape[0]
        wtile = flat.reshape(n // 16, 16).T.astype(np.int16)
        return np.tile(wtile, (8, 1))

    gidx_w = np.stack([wrap16(gidx[c]) for c in range(NCORES)])
    dcol_b = dcol.reshape(NCORES, total_slots // 128, 128).transpose(0, 2, 1).copy()

    return deg, gidx_w, dcol_b, call_S, call_start, o_list, total_slots


def _build_program(call_S, call_start, o_list, total_slots):
    from concourse import bass, bacc, mybir, tile

    f32 = mybir.dt.float32
    nc = bacc.Bacc(None, target_bir_lowering=False, num_swdge_queues=2)

    xT = nc.dram_tensor("xT", [FIN, NSH], f32, kind="ExternalInput")
    W1 = nc.dram_tensor("W1", [FIN, HID], f32, kind="ExternalInput")
    W2 = nc.dram_tensor("W2", [HID, FOUT], f32, kind="ExternalInput")
    b1 = nc.dram_tensor("b1", [HID, 1], f32, kind="ExternalInput")
    b2 = nc.dram_tensor("b2", [FOUT, 1], f32, kind="ExternalInput")
    degT = nc.dram_tensor("deg", [128, NTILE_NODE], f32, kind="ExternalInput")
    gidxT = nc.dram_tensor("gidx", [128, total_slots // 16], mybir.dt.int16,
                           kind="ExternalInput")
    dcolT = nc.dram_tensor("dcol", [128, total_slots // 128], f32,
                           kind="ExternalInput")
    outT = nc.dram_tensor("out", [FOUT, NSH], f32, kind="ExternalOutput")

    # per-layer, per-quarter staging + gathered tables
    tmy = [[nc.dram_tensor(f"tmy{l}_{q}", [QS[q], HID], f32) for q in range(NQ)]
           for l in (0, 1)]
    tfq = [[nc.dram_tensor(f"tfq{l}_{q}", [NCORES * QS[q], HID], f32,
                           addr_space="Shared") for q in range(NQ)]
           for l in (0, 1)]
    dinv_dram = nc.dram_tensor("dinv_dram", [NTILE_NODE * 128], f32)

    iota_np = np.tile(np.arange(128, dtype=np.float32), (128, 1))
    iota_dram = nc.inline_tensor(iota_np, name="iota128")
    ident_dram = nc.inline_tensor(np.eye(128, dtype=np.float32), name="ident128")

    rg = [list(range(NCORES))]

    def store_quarter(sbtab, dst_l, q):
        """DMA sbtab columns for quarter q into tmy[dst_l][q] (node-major)."""
        t0 = QB[q] // 128
        nfull = QS[q] // 128
        if nfull:
            nc.sync.dma_start(
                out=tmy[dst_l][q].ap()[:nfull * 128, :].rearrange(
                    "(t p) f -> p t f", p=128),
                in_=sbtab[:, (t0 * HID):(t0 + nfull) * HID].rearrange(
                    "p (t f) -> p t f", t=nfull))
        rem = QS[q] - nfull * 128
        if rem:
            nc.sync.dma_start(
                out=tmy[dst_l][q].ap()[nfull * 128:, :],
                in_=sbtab[:rem, (t0 + nfull) * HID:(t0 + nfull + 1) * HID])

    def allgather_quarter(dst_l, q):
        nc.gpsimd.collective_compute(
            "AllGather", mybir.AluOpType.bypass, replica_groups=rg,
            ins=[tmy[dst_l][q].ap().opt()], outs=[tfq[dst_l][q].ap().opt()])

    with tile.TileContext(nc) as tc:
        with (
            tc.tile_pool(name="const", bufs=1) as cpool,
            tc.tile_pool(name="dinvb", bufs=1) as dbpool,
            tc.tile_pool(name="stag2", bufs=1) as spool2,
            tc.tile_pool(name="psA", bufs=2, space="PSUM") as psA,
        ):
            w1t = cpool.tile([FIN, HID], f32)
            nc.sync.dma_start(out=w1t[:], in_=W1[:, :])
            w2t = cpool.tile([HID, FOUT], f32)
            nc.sync.dma_start(out=w2t[:], in_=W2[:, :])
            b1t = cpool.tile([HID, 1], f32)
            nc.sync.dma_start(out=b1t[:], in_=b1[:, :])
            b2t = cpool.tile([FOUT, 1], f32)
            nc.sync.dma_start(out=b2t[:], in_=b2[:, :])
            iot = cpool.tile([128, 128], f32)
            nc.sync.dma_start(out=iot[:], in_=iota_dram[:, :])
            idt = cpool.tile([128, 128], f32)
            nc.sync.dma_start(out=idt[:], in_=ident_dram[:, :])
            degt = cpool.tile([128, NTILE_NODE], f32)
            nc.sync.dma_start(out=degt[:], in_=degT[:, :])
            dsq = cpool.tile([128, NTILE_NODE], f32)
            nc.scalar.activation(dsq[:], degt[:],
                                 mybir.ActivationFunctionType.Sqrt)
            dinv = cpool.tile([128, NTILE_NODE], f32)
            nc.vector.reciprocal(dinv[:], dsq[:])
            nc.sync.dma_start(
                out=dinv_dram.ap().rearrange("(t p) -> p t", p=128), in_=dinv[:])
            dinvb = dbpool.tile([HID, NSH], f32)
            nc.sync.dma_start(out=dinvb[:1, :], in_=dinv_dram.ap()[None, :NSH])
            k = 1
            while k < HID:
                kk = min(k, HID - k)
                nc.sync.dma_start(out=dinvb[k:k + kk, :], in_=dinvb[:kk, :])
                k += kk

            sbTab2 = spool2.tile([128, NTILE_NODE * HID], f32)

            with tc.tile_pool(name="stag1", bufs=1) as spool1:
                sbTab1 = spool1.tile([128, NTILE_NODE * HID], f32)

                # ---- layer-1 table: tab1[n] = (x @ W1)[n] * dinv[n] ----
                XB = 8
                with tc.tile_pool(name="xT", bufs=2) as xpool:
                    for q in range(NQ):
                        tq0 = QB[q] // 128
                        tq1 = (QB[q] + QS[q] + 127) // 128
                        for t0 in range(tq0, tq1, XB):
                            t1 = min(tq1, t0 + XB)
                            n0, n1 = t0 * 128, min(NSH, t1 * 128)
                            xt = xpool.tile([FIN, XB * 128], f32)
                            nc.sync.dma_start(out=xt[:, :n1 - n0],
                                              in_=xT[:, n0:n1])
                            for t in range(t0, t1):
                                m0 = t * 128
                                nn = min(NSH, m0 + 128) - m0
                                sl = xt[:, (m0 - n0):(m0 - n0) + nn]
                                ps = psA.tile([128, HID], f32, space="PSUM")
                                nc.tensor.matmul(ps[:nn, :], lhsT=sl,
                                                 rhs=w1t[:],
                                                 start=True, stop=True)
                                nc.vector.tensor_scalar_mul(
                                    sbTab1[:nn, t * HID:(t + 1) * HID],
                                    ps[:nn, :], dinv[:nn, t:t + 1])
                        store_quarter(sbTab1, 0, q)
                        allgather_quarter(0, q)

                # ---- aggregation layers ----
                for layer in (0, 1):
                    sbtab = sbTab1 if layer == 0 else sbTab2
                    call_idx = 0
                    with (
                        tc.tile_pool(name=f"gb{layer}", bufs=4) as gpool,
                        tc.tile_pool(name=f"ix{layer}", bufs=4) as ipool,
                        tc.tile_pool(name=f"dc{layer}", bufs=4) as dpool,
                        tc.tile_pool(name=f"oh{layer}", bufs=4) as ohpool,
                        tc.tile_pool(name=f"fl{layer}", bufs=2) as flpool,
                        tc.tile_pool(name=f"ht{layer}", bufs=2) as htpool,
                        tc.tile_pool(name=f"psW{layer}", bufs=4,
                                     space="PSUM") as psW,
                    ):
                        for wg in range(NWG):
                            if layer == 0 and wg == 5:
                                allgather_quarter(1, 0)
                            if layer == 0 and wg == 8:
                                allgather_quarter(1, 1)
                            if layer == 0 and wg == 11:
                                allgather_quarter(1, 2)
                            ws = list(range(WGW * wg, min(WGW * wg + WGW, NW)))
                            psws = []
                            for w in ws:
                                p = psW.tile([HID, WIN], f32, space="PSUM")
                                nc.vector.memset(p[:], 0.0)
                                psws.append(p)
                                # self-loop term: psw[:, cols] += tab_tile^T
                                for k in range(4):
                                    t = 4 * w + k
                                    if t >= NTILE_NODE:
                                        break
                                    nn = min(NSH, t * 128 + 128) - t * 128
                                    nc.tensor.matmul(
                                        p[:, k * 128:k * 128 + nn],
                                        lhsT=sbtab[:nn,
                                                   t * HID:(t + 1) * HID],
                                        rhs=idt[:nn, :nn],
                                        start=False, stop=True)
                            for b in range(NQ):
                                gi = wg * NQ + b
                                S = int(call_S[wg, b])
                                if S == 0:
                                    continue
                                base = int(call_start[wg, b])
                                nb = S // 128
                                it = ipool.tile([128, S // 16],
                                                mybir.dt.int16)
                                nc.sync.dma_start(
                                    out=it[:],
                                    in_=gidxT[:, base // 16:
                                              base // 16 + S // 16])
                                dt_ = dpool.tile([128, nb], f32)
                                nc.sync.dma_start(
                                    out=dt_[:],
                                    in_=dcolT[:, base // 128:
                                              base // 128 + nb])
                                g = gpool.tile([128, nb * HID], f32)
                                nc.gpsimd.dma_gather(
                                    g[:].rearrange("p (n f) -> p n f", n=nb),
                                    tfq[layer][b][:, :],
                                    it[:], S, S, HID, single_packet=False,
                                    queue_num=call_idx % 2)
                                call_idx += 1
                                tiles = o_list[gi]
                                for j in range(nb):
                                    wi, offs = tiles[j]
                                    for (o, wd) in offs:
                                        oh = ohpool.tile([128, 128], f32)
                                        nc.vector.scalar_tensor_tensor(
                                            out=oh[:, :wd],
                                            in0=dt_[:, j:j + 1].to_broadcast(
                                                [128, wd]),
                                            scalar=float(o),
                                            in1=iot[:, :wd],
                                            op0=mybir.AluOpType.subtract,
                                            op1=mybir.AluOpType.is_equal)
                                        nc.tensor.matmul(
                                            psws[wi][:, o:o + wd],
                                            lhsT=g[:, j * HID:(j + 1) * HID],
                                            rhs=oh[:, :wd], start=False,
                                            stop=True)
                            # flush the group's windows
                            for wi, w in enumerate(ws):
                                c0 = w * WIN
                                c1 = min(NSH, c0 + WIN)
                                ncol = c1 - c0
                                if layer == 0:
                                    htr = htpool.tile([HID, WIN], f32)
                                    nc.vector.tensor_mul(htr[:, :ncol],
                                                         psws[wi][:, :ncol],
                                                         dinvb[:, c0:c1])
                                    nc.scalar.activation(
                                        htr[:, :ncol], htr[:, :ncol],
                                        mybir.ActivationFunctionType.Relu,
                                        bias=b1t[:])
                                    # layer-2 table tiles for this window
                                    for k in range(4):
                                        t = 4 * w + k
                                        if t >= NTILE_NODE:
                                            break
                                        nn = min(NSH, t * 128 + 128) - t * 128
                                        ps = psA.tile([128, FOUT], f32,
                                                      space="PSUM")
                                        nc.tensor.matmul(
                                            ps[:nn, :],
                                            lhsT=htr[:, k * 128:k * 128 + nn],
                                            rhs=w2t[:],
                                            start=True, stop=True)
                                        nc.vector.tensor_scalar_mul(
                                            sbTab2[:nn,
                                                   t * HID:(t + 1) * HID],
                                            ps[:nn, :], dinv[:nn, t:t + 1])
                                else:
                                    fl = flpool.tile([HID, WIN], f32)
                                    nc.vector.tensor_mul(fl[:, :ncol],
                                                         psws[wi][:, :ncol],
                                                         dinvb[:, c0:c1])
                                    nc.vector.tensor_scalar_add(
                                        fl[:, :ncol], fl[:, :ncol], b2t[:])
                                    nc.sync.dma_start(out=outT[:, c0:c1],
                                                      in_=fl[:, :ncol])
                            if layer == 0 and wg in (3, 6, 9, 12):
                                # quarter fully flushed -> stage layer-2 rows
                                store_quarter(sbTab2, 1, {3: 0, 6: 1,
                                                          9: 2, 12: 3}[wg])
                        if layer == 0:
                            allgather_quarter(1, 3)
    nc.compile()
    return nc


TRACE = False        # set True (e.g. from test.py) to capture HW exec time
_LAST_TIMING = None


def kernel(x, edge_index, W1, b1, W2, b2):
    from concourse.bass_utils import run_bass_kernel_spmd

    x = np.asarray(x, np.float32)
    W1 = np.asarray(W1, np.float32)
    W2 = np.asarray(W2, np.float32)
    b1 = np.asarray(b1, np.float32)
    b2 = np.asarray(b2, np.float32)

    deg, gidx_w, dcol_b, call_S, call_start, o_list, total_slots = \
        _preprocess(edge_index)

    nc = _build_program(call_S, call_start, o_list, total_slots)

    in_maps = []
    for c in range(NCORES):
        lo, hi = c * NSH, (c + 1) * NSH
        degc = deg[lo:hi]
        degp = np.ones(NTILE_NODE * 128, np.float32)
        degp[:NSH] = degc
        in_maps.append({
            "xT": np.ascontiguousarray(x[lo:hi].T),
            "W1": W1, "W2": W2,
            "b1": b1.reshape(HID, 1), "b2": b2.reshape(FOUT, 1),
            "deg": np.ascontiguousarray(degp.reshape(NTILE_NODE, 128).T),
            "gidx": gidx_w[c],
            "dcol": dcol_b[c],
        })

    kwargs = {"trace": True} if TRACE else {}
    res = run_bass_kernel_spmd(nc, in_maps, core_ids=list(range(NCORES)),
                               **kwargs)
    globals()["_LAST_TIMING"] = getattr(res, "exec_time_ns", None)

    z = np.empty((N, FOUT), np.float32)
    for c in range(NCORES):
        lo, hi = c * NSH, (c + 1) * NSH
        z[lo:hi] = np.asarray(res.results[c]["out"]).reshape(FOUT, NSH).T
    return z
